# revision 9
# baseline (speedup 1.0000x reference)
"""Entity-knowledge embedding lookup kernel for Trainium2 (8 NeuronCores).

Math: for each token t (B*L=4096 total) with 8 labels,
    y[t] = conv_w @ mean_{j,k}(fact_table[label[t,j]] viewed as [16,300]) + conv_b
The mean over the 128 (8 labels x 16 subvectors) rows commutes with the 1x1
conv, so the kernel is: gather 8 fact rows per token, reduce 4800->300 on
DVE, then a tiny matmul per 128-token group.

Sharding: data-parallel over tokens — 512 tokens per core; fact table and
conv weights replicated (kernel() uses build_nc_bypass).

Alternative two-phase builders (build_nc_v2/v3/v4) cut per-core HBM traffic
from 78.6MB to ~50MB by precomputing G[v] = (conv_w/128) @ sum_k fact[v,k,:]
vocab-sharded (48MB sequential read per core), AllGather-ing the 8MB G table,
and doing per-token 400B-row gathers. They are numerically correct
(rel err ~2e-7) but the 8-core DRAM AllGather on this axon terminal costs
~1.1ms per call in most measurement windows, which erases the traffic win
for a single-shot call; build_nc_v4 software-pipelines the AllGather across
loop iterations (phase 2 of iteration i reads the table gathered in i-1,
legal because G is call-invariant) for steady-state throughput. Measured
paired-median per-iteration times are within noise of bypass, so the
replicated-gather kernel stays the default.
"""

import sys

import numpy as np

sys.path.insert(0, "/opt/trn_rl_repo")

import concourse.bacc as bacc
import concourse.bass as bass
import concourse.mybir as mybir
from concourse.masks import make_identity
from concourse.tile import TileContext

VOCAB = 20000
TOPK = 8
GLOVE = 300
OUTC = 100
B, L, NL = 32, 128, 8
NCORES = 8
TOKENS = B * L            # 4096
TPC = TOKENS // NCORES    # 512 tokens per core
GROUP = 128               # tokens per SBUF tile group
NGROUPS = TPC // GROUP    # 4
ROW = 2 * TOPK * GLOVE    # 4800 floats per fact row
NCHUNK = 3                # CCE add maxes at 2048 elems; 4800/3 = 1600
CH = ROW // NCHUNK        # 1600 elements per gather chunk

F32 = mybir.dt.float32
BF16 = mybir.dt.bfloat16
I32 = mybir.dt.int32


def build_nc(loops=1):
    nc = bacc.Bacc("TRN2", target_bir_lowering=False, debug=False)

    fact = nc.dram_tensor("fact", [VOCAB, ROW], F32, kind="ExternalInput").ap()
    labels = nc.dram_tensor("labels", [TPC, NL], I32, kind="ExternalInput").ap()
    # conv_w.T pre-scaled by 1/128 on host: [300, 100]
    wt = nc.dram_tensor("wt", [GLOVE, OUTC], F32, kind="ExternalInput").ap()
    bias = nc.dram_tensor("bias", [OUTC, 1], F32, kind="ExternalInput").ap()
    # output transposed: [100, 512]; host transposes back
    y = nc.dram_tensor("y", [OUTC, TPC], F32, kind="ExternalOutput").ap()

    with TileContext(nc) as tc:
        with (
            tc.tile_pool(name="const", bufs=1) as cpool,
            tc.tile_pool(name="acc", bufs=4) as apool,
            tc.tile_pool(name="small", bufs=4) as spool,
            tc.tile_pool(name="ps_t", bufs=3, space="PSUM") as ppool_t,
            tc.tile_pool(name="ps_y", bufs=2, space="PSUM") as ppool_y,
        ):
            # constants are DVE-copied so PE instructions depend only on the
            # DVE semaphore (PE allows a single sync-wait slot on TRN2)
            ident0 = cpool.tile([128, 128], F32, tag="ident0")
            make_identity(nc, ident0[:])
            ident = cpool.tile([128, 128], F32, tag="ident")
            nc.vector.tensor_copy(ident[:], ident0[:])
            wts = []
            for k in range(3):
                t0 = cpool.tile([100, OUTC], F32, tag=f"wt{k}raw")
                nc.sync.dma_start(out=t0[:], in_=wt[k * 100 : (k + 1) * 100, :])
                t = cpool.tile([100, OUTC], F32, tag=f"wt{k}")
                nc.vector.tensor_copy(t[:], t0[:])
                wts.append(t)
            btile = cpool.tile([OUTC, 1], F32)
            nc.sync.dma_start(out=btile[:], in_=bias[:])

            for _ in range(loops):
                # phase A: per group, load indices + bypass gather (j=0);
                # phase B: accumulating gathers interleaved j-outer so the
                # in-order SWDGE queue never stalls on a dependent chain link
                idxs, accs = [], []
                for g in range(NGROUPS):
                    tok0 = g * GROUP
                    idx = spool.tile([GROUP, NL], I32, tag="idx")
                    nc.sync.dma_start(out=idx[:], in_=labels[tok0 : tok0 + GROUP, :])
                    idxs.append(idx)
                    acc = apool.tile([GROUP, ROW], F32, tag="acc")
                    nc.gpsimd.indirect_dma_start(
                        out=acc[:],
                        out_offset=None,
                        in_=fact[:],
                        in_offset=bass.IndirectOffsetOnAxis(ap=idx[:, 0:1], axis=0),
                        compute_op=mybir.AluOpType.bypass,
                    )
                    accs.append(acc)
                # CCE add maxes at 2048 elements per descriptor -> 3 chunks
                for j in range(1, NL):
                    for g in range(NGROUPS):
                        for k in range(NCHUNK):
                            nc.gpsimd.indirect_dma_start(
                                out=accs[g][:, k * CH : (k + 1) * CH],
                                out_offset=None,
                                in_=fact[:],
                                in_offset=bass.IndirectOffsetOnAxis(
                                    ap=idxs[g][:, j : j + 1], axis=0
                                ),
                                element_offset=k * CH,
                                compute_op=mybir.AluOpType.add,
                            )

                for g in range(NGROUPS):
                    acc = accs[g]
                    tok0 = g * GROUP
                    # tree-reduce free dim 4800 -> 600 in place, final step
                    # into a fresh DVE-only tile (keeps PE at 1 sem wait)
                    w = ROW
                    while w > 2 * GLOVE:
                        h = w // 2
                        nc.vector.tensor_add(acc[:, :h], acc[:, :h], acc[:, h:w])
                        w = h
                    s = spool.tile([GROUP, GLOVE], F32, tag="s")
                    nc.vector.tensor_add(
                        s[:], acc[:, :GLOVE], acc[:, GLOVE : 2 * GLOVE]
                    )

                    # conv: out[o,t] = sum_c w[o,c]/128 * s[t,c] (3 c-chunks)
                    yt = spool.tile([OUTC, GROUP], F32, tag="yt")
                    nc.vector.tensor_copy(yt[:], btile[:].to_broadcast([OUTC, GROUP]))
                    for k in range(3):
                        tp = ppool_t.tile([100, GROUP], F32, tag="tp")
                        nc.tensor.transpose(
                            out=tp[:],
                            in_=s[:, k * 100 : (k + 1) * 100],
                            identity=ident[:],
                        )
                        st = spool.tile([100, GROUP], F32, tag="st")
                        nc.vector.tensor_copy(st[:], tp[:])
                        yp = ppool_y.tile([OUTC, GROUP], F32, tag="yp")
                        nc.tensor.matmul(
                            yp[:], wts[k][:], st[:], start=True, stop=True
                        )
                        nc.vector.tensor_add(yt[:], yt[:], yp[:])
                    nc.sync.dma_start(out=y[:, tok0 : tok0 + GROUP], in_=yt[:])

    nc.finalize()
    return nc


def build_nc_bypass(loops=1):
    """All-bypass variant: per (group, label) gather [128 tokens, 4800] with
    plain bypass (no DMA CCE), DVE tree-reduce each to [128, 300], and
    DVE-accumulate the 8 labels into the group sum. Same inputs as v1."""
    nc = bacc.Bacc("TRN2", target_bir_lowering=False, debug=False)

    fact = nc.dram_tensor("fact", [VOCAB, ROW], F32, kind="ExternalInput").ap()
    labels = nc.dram_tensor("labels", [TPC, NL], I32, kind="ExternalInput").ap()
    wt = nc.dram_tensor("wt", [GLOVE, OUTC], F32, kind="ExternalInput").ap()
    bias = nc.dram_tensor("bias", [OUTC, 1], F32, kind="ExternalInput").ap()
    y = nc.dram_tensor("y", [OUTC, TPC], F32, kind="ExternalOutput").ap()

    with TileContext(nc) as tc:
        with (
            tc.tile_pool(name="const", bufs=1) as cpool,
            tc.tile_pool(name="acc", bufs=6) as apool,
            tc.tile_pool(name="small", bufs=4) as spool,
            tc.tile_pool(name="ssum", bufs=3) as sspool,
            tc.tile_pool(name="ps_t", bufs=3, space="PSUM") as ppool_t,
            tc.tile_pool(name="ps_y", bufs=2, space="PSUM") as ppool_y,
        ):
            ident0 = cpool.tile([128, 128], F32, tag="ident0")
            make_identity(nc, ident0[:])
            ident = cpool.tile([128, 128], F32, tag="ident")
            nc.vector.tensor_copy(ident[:], ident0[:])
            wts = []
            for k in range(3):
                t0 = cpool.tile([100, OUTC], F32, tag=f"wt{k}raw")
                nc.sync.dma_start(out=t0[:], in_=wt[k * 100 : (k + 1) * 100, :])
                t = cpool.tile([100, OUTC], F32, tag=f"wt{k}")
                nc.vector.tensor_copy(t[:], t0[:])
                wts.append(t)
            btile = cpool.tile([OUTC, 1], F32)
            nc.sync.dma_start(out=btile[:], in_=bias[:])

            for _ in range(loops):
                for g in range(NGROUPS):
                    tok0 = g * GROUP
                    idx = spool.tile([GROUP, NL], I32, tag="idx")
                    nc.sync.dma_start(out=idx[:], in_=labels[tok0 : tok0 + GROUP, :])
                    sj_all = sspool.tile([GROUP, NL * GLOVE], F32, tag="sj_all")
                    for j in range(NL):
                        acc = apool.tile([GROUP, ROW], F32, tag="acc")
                        nc.gpsimd.indirect_dma_start(
                            out=acc[:],
                            out_offset=None,
                            in_=fact[:],
                            in_offset=bass.IndirectOffsetOnAxis(
                                ap=idx[:, j : j + 1], axis=0
                            ),
                            compute_op=mybir.AluOpType.bypass,
                        )
                        # one strided reduce: sum the 16 subvectors per row
                        nc.vector.tensor_reduce(
                            out=sj_all[:, j * GLOVE : (j + 1) * GLOVE],
                            in_=acc[:].rearrange("p (k c) -> p c k", k=2 * TOPK),
                            axis=mybir.AxisListType.X,
                            op=mybir.AluOpType.add,
                        )
                    ssum = sspool.tile([GROUP, GLOVE], F32, tag="ssum")
                    nc.vector.tensor_reduce(
                        out=ssum[:],
                        in_=sj_all[:].rearrange("p (j c) -> p c j", j=NL),
                        axis=mybir.AxisListType.X,
                        op=mybir.AluOpType.add,
                    )

                    yt = spool.tile([OUTC, GROUP], F32, tag="yt")
                    nc.vector.tensor_copy(yt[:], btile[:].to_broadcast([OUTC, GROUP]))
                    for k in range(3):
                        tp = ppool_t.tile([100, GROUP], F32, tag="tp")
                        nc.tensor.transpose(
                            out=tp[:],
                            in_=ssum[:, k * 100 : (k + 1) * 100],
                            identity=ident[:],
                        )
                        st = spool.tile([100, GROUP], F32, tag="st")
                        nc.vector.tensor_copy(st[:], tp[:])
                        yp = ppool_y.tile([OUTC, GROUP], F32, tag="yp")
                        nc.tensor.matmul(
                            yp[:], wts[k][:], st[:], start=True, stop=True
                        )
                        nc.vector.tensor_add(yt[:], yt[:], yp[:])
                    nc.sync.dma_start(out=y[:, tok0 : tok0 + GROUP], in_=yt[:])

    nc.finalize()
    return nc


VSHARD = VOCAB // NCORES  # 2500 vocab rows per core
VTILE = 128
NVT = (VSHARD + VTILE - 1) // VTILE  # 20 tiles (last = 68 rows)


def build_nc_v2(loops=1, phase1=True, ag=True, phase2=True):
    """Two-phase: (1) vocab-sharded precompute of G[v] = (conv_w/128) @
    sum_k fact[v,k,:]  -> [2500,100] per core, AllGather to full [20000,100];
    (2) per-token gather of 400B G rows + reduce over 8 labels + bias.
    Cuts per-core HBM traffic from 78.6MB (full-row gathers) to ~50MB
    (one sequential pass over the table shard + tiny gathers)."""
    nc = bacc.Bacc("TRN2", target_bir_lowering=False, debug=False)

    fact = nc.dram_tensor("fact", [VSHARD, ROW], F32, kind="ExternalInput").ap()
    labels = nc.dram_tensor("labels", [TPC, NL], I32, kind="ExternalInput").ap()
    # conv_w.T pre-scaled by 1/128 on host: [300, 100]
    wt = nc.dram_tensor("wt", [GLOVE, OUTC], F32, kind="ExternalInput").ap()
    # bias replicated to all 128 partitions on host: [128, 100]
    biasr = nc.dram_tensor("biasr", [128, OUTC], F32, kind="ExternalInput").ap()
    y = nc.dram_tensor("y", [TPC, OUTC], F32, kind="ExternalOutput").ap()

    with TileContext(nc) as tc:
        with (
            tc.tile_pool(name="const", bufs=1) as cpool,
            tc.tile_pool(name="facts", bufs=3) as fpool,
            tc.tile_pool(name="small", bufs=4) as spool,
            tc.tile_pool(name="gath", bufs=3) as gpool,
            tc.tile_pool(name="ps_t", bufs=3, space="PSUM") as ppool_t,
            tc.tile_pool(name="ps_g", bufs=2, space="PSUM") as ppool_g,
            tc.tile_pool(name="dram", bufs=2, space="DRAM") as dpool,
        ):
            ident0 = cpool.tile([128, 128], F32, tag="ident0")
            make_identity(nc, ident0[:])
            ident = cpool.tile([128, 128], F32, tag="ident")
            nc.vector.tensor_copy(ident[:], ident0[:])
            wts = []
            for k in range(3):
                t0 = cpool.tile([100, OUTC], F32, tag=f"wt{k}raw")
                nc.sync.dma_start(out=t0[:], in_=wt[k * 100 : (k + 1) * 100, :])
                t = cpool.tile([100, OUTC], F32, tag=f"wt{k}")
                nc.vector.tensor_copy(t[:], t0[:])
                wts.append(t)
            btile = cpool.tile([128, OUTC], F32, tag="biasr")
            nc.sync.dma_start(out=btile[:], in_=biasr[:])

            for _ in range(loops):
                gshard = dpool.tile([VSHARD, OUTC], F32, tag="gshard")
                gfull = dpool.tile([VOCAB, OUTC], F32, tag="gfull")

                # ---- phase 1: G shard = (sum_k fact[v,k,:]) @ wt ----
                for t in range(NVT if phase1 else 0):
                    v0 = t * VTILE
                    rows = min(VTILE, VSHARD - v0)
                    ft = fpool.tile([VTILE, ROW], F32, tag="ft")
                    nc.sync.dma_start(out=ft[:rows], in_=fact[v0 : v0 + rows, :])
                    s = spool.tile([VTILE, GLOVE], F32, tag="s")
                    nc.vector.tensor_reduce(
                        out=s[:rows],
                        in_=ft[:rows].rearrange("p (k c) -> p c k", k=2 * TOPK),
                        axis=mybir.AxisListType.X,
                        op=mybir.AluOpType.add,
                    )
                    gp = ppool_g.tile([VTILE, OUTC], F32, tag="gp")
                    for k in range(3):
                        tp = ppool_t.tile([100, VTILE], F32, tag="tp")
                        nc.tensor.transpose(
                            out=tp[:, :rows],
                            in_=s[:rows, k * 100 : (k + 1) * 100],
                            identity=ident[:rows, :rows],
                        )
                        st = spool.tile([100, VTILE], F32, tag="st")
                        nc.vector.tensor_copy(st[:, :rows], tp[:, :rows])
                        nc.tensor.matmul(
                            gp[:rows],
                            st[:, :rows],
                            wts[k][:],
                            start=(k == 0),
                            stop=(k == 2),
                        )
                    gs = spool.tile([VTILE, OUTC], F32, tag="gs")
                    nc.vector.tensor_copy(gs[:rows], gp[:rows])
                    nc.sync.dma_start(out=gshard[v0 : v0 + rows, :], in_=gs[:rows])

                # ---- allgather shards -> full G table ----
                if ag:
                    nc.gpsimd.collective_compute(
                        "AllGather",
                        mybir.AluOpType.bypass,
                        replica_groups=[list(range(NCORES))],
                        ins=[gshard[:]],
                        outs=[gfull[:]],
                    )

                # ---- phase 2: per-token gather + label-reduce + bias ----
                for g in range(NGROUPS if phase2 else 0):
                    tok0 = g * GROUP
                    idx = spool.tile([GROUP, NL], I32, tag="idx")
                    nc.sync.dma_start(out=idx[:], in_=labels[tok0 : tok0 + GROUP, :])
                    fac = gpool.tile([GROUP, NL * OUTC], F32, tag="fac")
                    for j in range(NL):
                        nc.gpsimd.indirect_dma_start(
                            out=fac[:, j * OUTC : (j + 1) * OUTC],
                            out_offset=None,
                            in_=gfull[:],
                            in_offset=bass.IndirectOffsetOnAxis(
                                ap=idx[:, j : j + 1], axis=0
                            ),
                            compute_op=mybir.AluOpType.bypass,
                        )
                    yt = spool.tile([GROUP, OUTC], F32, tag="yt")
                    nc.vector.tensor_reduce(
                        out=yt[:],
                        in_=fac[:].rearrange("p (j c) -> p c j", j=NL),
                        axis=mybir.AxisListType.X,
                        op=mybir.AluOpType.add,
                    )
                    nc.vector.tensor_add(yt[:], yt[:], btile[:])
                    nc.sync.dma_start(out=y[tok0 : tok0 + GROUP, :], in_=yt[:])

    nc.finalize()
    return nc


NCH4 = 4                    # column chunks per v-tile load
CHW = ROW // NCH4           # 1200 floats (4 k-blocks) per chunk


def build_nc_v3(loops=1, phase1=True, ag=True, phase2=True):
    """v2 with phase-1 loads split into 4 column-chunk DMAs per v-tile and a
    16-buffer chunk pool, so >=16 DMA transfers are in flight at once (one
    dma_start rides one DMA engine; concurrency = aggregate bandwidth).
    AllGather output in Shared DRAM (fast path)."""
    nc = bacc.Bacc("TRN2", target_bir_lowering=False, debug=False)

    fact = nc.dram_tensor("fact", [VSHARD, ROW], F32, kind="ExternalInput").ap()
    labels = nc.dram_tensor("labels", [TPC, NL], I32, kind="ExternalInput").ap()
    wt = nc.dram_tensor("wt", [GLOVE, OUTC], F32, kind="ExternalInput").ap()
    biasr = nc.dram_tensor("biasr", [128, OUTC], F32, kind="ExternalInput").ap()
    y = nc.dram_tensor("y", [TPC, OUTC], F32, kind="ExternalOutput").ap()

    gshards, gfulls = [], []
    for i in range(2):
        gshards.append(
            nc.dram_tensor(f"gshard{i}", [VSHARD, OUTC], F32, kind="Internal").ap()
        )
        gfulls.append(
            nc.dram_tensor(
                f"gfull{i}", [VOCAB, OUTC], F32, kind="Internal", addr_space="Shared"
            ).ap()
        )

    with TileContext(nc) as tc:
        with (
            tc.tile_pool(name="const", bufs=1) as cpool,
            tc.tile_pool(name="chunk", bufs=16) as kpool,
            tc.tile_pool(name="s4", bufs=3) as s4pool,
            tc.tile_pool(name="small", bufs=4) as spool,
            tc.tile_pool(name="gath", bufs=3) as gpool,
            tc.tile_pool(name="ps_t", bufs=3, space="PSUM") as ppool_t,
            tc.tile_pool(name="ps_g", bufs=2, space="PSUM") as ppool_g,
        ):
            ident0 = cpool.tile([128, 128], F32, tag="ident0")
            make_identity(nc, ident0[:])
            ident = cpool.tile([128, 128], F32, tag="ident")
            nc.vector.tensor_copy(ident[:], ident0[:])
            wts = []
            for k in range(3):
                t0 = cpool.tile([100, OUTC], F32, tag=f"wt{k}raw")
                nc.sync.dma_start(out=t0[:], in_=wt[k * 100 : (k + 1) * 100, :])
                t = cpool.tile([100, OUTC], F32, tag=f"wt{k}")
                nc.vector.tensor_copy(t[:], t0[:])
                wts.append(t)
            btile = cpool.tile([128, OUTC], F32, tag="biasr")
            nc.sync.dma_start(out=btile[:], in_=biasr[:])

            for it in range(loops):
                gshard = gshards[it % 2]
                gfull = gfulls[it % 2]

                # ---- phase 1 ----
                for t in range(NVT if phase1 else 0):
                    v0 = t * VTILE
                    rows = min(VTILE, VSHARD - v0)
                    s4 = s4pool.tile([VTILE, NCH4 * GLOVE], F32, tag="s4")
                    for c in range(NCH4):
                        ck = kpool.tile([VTILE, CHW], F32, tag="ck")
                        nc.sync.dma_start(
                            out=ck[:rows],
                            in_=fact[v0 : v0 + rows, c * CHW : (c + 1) * CHW],
                        )
                        nc.vector.tensor_reduce(
                            out=s4[:rows, c * GLOVE : (c + 1) * GLOVE],
                            in_=ck[:rows].rearrange(
                                "p (k c) -> p c k", k=NCH4
                            ),
                            axis=mybir.AxisListType.X,
                            op=mybir.AluOpType.add,
                        )
                    s = spool.tile([VTILE, GLOVE], F32, tag="s")
                    nc.vector.tensor_reduce(
                        out=s[:rows],
                        in_=s4[:rows].rearrange("p (q c) -> p c q", q=NCH4),
                        axis=mybir.AxisListType.X,
                        op=mybir.AluOpType.add,
                    )
                    gp = ppool_g.tile([VTILE, OUTC], F32, tag="gp")
                    for k in range(3):
                        tp = ppool_t.tile([100, VTILE], F32, tag="tp")
                        nc.tensor.transpose(
                            out=tp[:, :rows],
                            in_=s[:rows, k * 100 : (k + 1) * 100],
                            identity=ident[:rows, :rows],
                        )
                        st = spool.tile([100, VTILE], F32, tag="st")
                        nc.vector.tensor_copy(st[:, :rows], tp[:, :rows])
                        nc.tensor.matmul(
                            gp[:rows],
                            st[:, :rows],
                            wts[k][:],
                            start=(k == 0),
                            stop=(k == 2),
                        )
                    gs = spool.tile([VTILE, OUTC], F32, tag="gs")
                    nc.vector.tensor_copy(gs[:rows], gp[:rows])
                    nc.sync.dma_start(out=gshard[v0 : v0 + rows, :], in_=gs[:rows])

                # ---- allgather ----
                if ag:
                    nc.gpsimd.collective_compute(
                        "AllGather",
                        mybir.AluOpType.bypass,
                        replica_groups=[list(range(NCORES))],
                        ins=[gshard[:, :]],
                        outs=[gfull[:, :]],
                    )

                # ---- phase 2 ----
                for g in range(NGROUPS if phase2 else 0):
                    tok0 = g * GROUP
                    idx = spool.tile([GROUP, NL], I32, tag="idx")
                    nc.sync.dma_start(out=idx[:], in_=labels[tok0 : tok0 + GROUP, :])
                    fac = gpool.tile([GROUP, NL * OUTC], F32, tag="fac")
                    for j in range(NL):
                        nc.gpsimd.indirect_dma_start(
                            out=fac[:, j * OUTC : (j + 1) * OUTC],
                            out_offset=None,
                            in_=gfull[:, :],
                            in_offset=bass.IndirectOffsetOnAxis(
                                ap=idx[:, j : j + 1], axis=0
                            ),
                            compute_op=mybir.AluOpType.bypass,
                        )
                    yt = spool.tile([GROUP, OUTC], F32, tag="yt")
                    nc.vector.tensor_reduce(
                        out=yt[:],
                        in_=fac[:].rearrange("p (j c) -> p c j", j=NL),
                        axis=mybir.AxisListType.X,
                        op=mybir.AluOpType.add,
                    )
                    nc.vector.tensor_add(yt[:], yt[:], btile[:])
                    nc.sync.dma_start(out=y[tok0 : tok0 + GROUP, :], in_=yt[:])

    nc.finalize()
    return nc


def build_nc_v4(loops=1, gdtype=None):
    """v3 with the AllGather software-pipelined across loop iterations:
    phase 2 of iteration i consumes the gfull produced by iteration i-1
    (identical values -- G only depends on the call inputs), so the
    collective's latency is off the steady-state critical path. Iteration 0
    waits for its own AllGather, so a loops=1 build is the plain serial
    phase1 -> allgather -> phase2 kernel. gdtype=BF16 halves the G table
    (allgather bytes + gather row size) at ~2e-3 relative error."""
    if gdtype is None:
        gdtype = F32
    nc = bacc.Bacc("TRN2", target_bir_lowering=False, debug=False)

    fact = nc.dram_tensor("fact", [VSHARD, ROW], F32, kind="ExternalInput").ap()
    labels = nc.dram_tensor("labels", [TPC, NL], I32, kind="ExternalInput").ap()
    wt = nc.dram_tensor("wt", [GLOVE, OUTC], F32, kind="ExternalInput").ap()
    biasr = nc.dram_tensor("biasr", [128, OUTC], F32, kind="ExternalInput").ap()
    y = nc.dram_tensor("y", [TPC, OUTC], F32, kind="ExternalOutput").ap()

    gshards, gfulls = [], []
    for i in range(2):
        gshards.append(
            nc.dram_tensor(f"gshard{i}", [VSHARD, OUTC], gdtype, kind="Internal").ap()
        )
        gfulls.append(
            nc.dram_tensor(
                f"gfull{i}", [VOCAB, OUTC], gdtype, kind="Internal", addr_space="Shared"
            ).ap()
        )

    with TileContext(nc) as tc:
        with (
            tc.tile_pool(name="const", bufs=1) as cpool,
            tc.tile_pool(name="chunk", bufs=16) as kpool,
            tc.tile_pool(name="s4", bufs=3) as s4pool,
            tc.tile_pool(name="small", bufs=4) as spool,
            tc.tile_pool(name="gath", bufs=3) as gpool,
            tc.tile_pool(name="ps_t", bufs=3, space="PSUM") as ppool_t,
            tc.tile_pool(name="ps_g", bufs=2, space="PSUM") as ppool_g,
        ):
            ident0 = cpool.tile([128, 128], F32, tag="ident0")
            make_identity(nc, ident0[:])
            ident = cpool.tile([128, 128], F32, tag="ident")
            nc.vector.tensor_copy(ident[:], ident0[:])
            wts = []
            for k in range(3):
                t0 = cpool.tile([100, OUTC], F32, tag=f"wt{k}raw")
                nc.sync.dma_start(out=t0[:], in_=wt[k * 100 : (k + 1) * 100, :])
                t = cpool.tile([100, OUTC], F32, tag=f"wt{k}")
                nc.vector.tensor_copy(t[:], t0[:])
                wts.append(t)
            btile = cpool.tile([128, OUTC], F32, tag="biasr")
            nc.sync.dma_start(out=btile[:], in_=biasr[:])

            for it in range(loops):
                gshard = gshards[it % 2]
                gfull = gfulls[it % 2]
                # phase 2 of iteration it reads the table gathered in it-1
                gread = gfulls[(it - 1) % 2] if it > 0 else gfulls[0]

                # ---- phase 1 ----
                for t in range(NVT):
                    v0 = t * VTILE
                    rows = min(VTILE, VSHARD - v0)
                    s4 = s4pool.tile([VTILE, NCH4 * GLOVE], F32, tag="s4")
                    for c in range(NCH4):
                        ck = kpool.tile([VTILE, CHW], F32, tag="ck")
                        nc.sync.dma_start(
                            out=ck[:rows],
                            in_=fact[v0 : v0 + rows, c * CHW : (c + 1) * CHW],
                        )
                        nc.vector.tensor_reduce(
                            out=s4[:rows, c * GLOVE : (c + 1) * GLOVE],
                            in_=ck[:rows].rearrange("p (k c) -> p c k", k=NCH4),
                            axis=mybir.AxisListType.X,
                            op=mybir.AluOpType.add,
                        )
                    s = spool.tile([VTILE, GLOVE], F32, tag="s")
                    nc.vector.tensor_reduce(
                        out=s[:rows],
                        in_=s4[:rows].rearrange("p (q c) -> p c q", q=NCH4),
                        axis=mybir.AxisListType.X,
                        op=mybir.AluOpType.add,
                    )
                    gp = ppool_g.tile([VTILE, OUTC], F32, tag="gp")
                    for k in range(3):
                        tp = ppool_t.tile([100, VTILE], F32, tag="tp")
                        nc.tensor.transpose(
                            out=tp[:, :rows],
                            in_=s[:rows, k * 100 : (k + 1) * 100],
                            identity=ident[:rows, :rows],
                        )
                        st = spool.tile([100, VTILE], F32, tag="st")
                        nc.vector.tensor_copy(st[:, :rows], tp[:, :rows])
                        nc.tensor.matmul(
                            gp[:rows],
                            st[:, :rows],
                            wts[k][:],
                            start=(k == 0),
                            stop=(k == 2),
                        )
                    gs = spool.tile([VTILE, OUTC], gdtype, tag="gs")
                    nc.vector.tensor_copy(gs[:rows], gp[:rows])
                    nc.sync.dma_start(out=gshard[v0 : v0 + rows, :], in_=gs[:rows])

                # ---- allgather (consumed by NEXT iteration's phase 2) ----
                nc.gpsimd.collective_compute(
                    "AllGather",
                    mybir.AluOpType.bypass,
                    replica_groups=[list(range(NCORES))],
                    ins=[gshard[:, :]],
                    outs=[gfull[:, :]],
                )

                # ---- phase 2 ----
                for g in range(NGROUPS):
                    tok0 = g * GROUP
                    idx = spool.tile([GROUP, NL], I32, tag="idx")
                    nc.sync.dma_start(out=idx[:], in_=labels[tok0 : tok0 + GROUP, :])
                    fac = gpool.tile([GROUP, NL * OUTC], gdtype, tag="fac")
                    for j in range(NL):
                        nc.gpsimd.indirect_dma_start(
                            out=fac[:, j * OUTC : (j + 1) * OUTC],
                            out_offset=None,
                            in_=gread[:, :],
                            in_offset=bass.IndirectOffsetOnAxis(
                                ap=idx[:, j : j + 1], axis=0
                            ),
                            compute_op=mybir.AluOpType.bypass,
                        )
                    yt = spool.tile([GROUP, OUTC], F32, tag="yt")
                    nc.vector.tensor_reduce(
                        out=yt[:],
                        in_=fac[:].rearrange("p (j c) -> p c j", j=NL),
                        axis=mybir.AxisListType.X,
                        op=mybir.AluOpType.add,
                    )
                    nc.vector.tensor_add(yt[:], yt[:], btile[:])
                    nc.sync.dma_start(out=y[tok0 : tok0 + GROUP, :], in_=yt[:])

    nc.finalize()
    return nc


def make_in_maps_v2(detect_labels, fact_table, conv_w, conv_b):
    labels_flat = np.ascontiguousarray(
        detect_labels.reshape(TOKENS, NL).astype(np.int32)
    )
    fact2d = np.ascontiguousarray(fact_table.reshape(VOCAB, ROW).astype(np.float32))
    wt = np.ascontiguousarray(conv_w.T.astype(np.float32) / 128.0)
    biasr = np.ascontiguousarray(
        np.tile(conv_b.astype(np.float32).reshape(1, OUTC), (128, 1))
    )
    in_maps = []
    for c in range(NCORES):
        in_maps.append(
            {
                "fact": np.ascontiguousarray(fact2d[c * VSHARD : (c + 1) * VSHARD]),
                "labels": np.ascontiguousarray(labels_flat[c * TPC : (c + 1) * TPC]),
                "wt": wt,
                "biasr": biasr,
            }
        )
    return in_maps


def assemble_output_v2(results):
    # results: list of per-core dicts with "y" [512, 100]
    parts = [np.asarray(r["y"]) for r in results]
    return np.concatenate(parts, axis=0).reshape(B, L, OUTC).astype(np.float32)


def make_in_maps(detect_labels, fact_table, conv_w, conv_b):
    labels_flat = np.ascontiguousarray(
        detect_labels.reshape(TOKENS, NL).astype(np.int32)
    )
    fact2d = np.ascontiguousarray(fact_table.reshape(VOCAB, ROW).astype(np.float32))
    wt = np.ascontiguousarray(conv_w.T.astype(np.float32) / 128.0)
    bias2d = np.ascontiguousarray(conv_b.astype(np.float32).reshape(OUTC, 1))
    in_maps = []
    for c in range(NCORES):
        in_maps.append(
            {
                "fact": fact2d,
                "labels": np.ascontiguousarray(labels_flat[c * TPC : (c + 1) * TPC]),
                "wt": wt,
                "bias": bias2d,
            }
        )
    return in_maps


def assemble_output(results):
    # results: list of per-core dicts with "y" [100, 512]
    parts = [np.asarray(r["y"]).T for r in results]  # each [512, 100]
    return np.concatenate(parts, axis=0).reshape(B, L, OUTC).astype(np.float32)


def kernel(detect_labels, fact_table, conv_w, conv_b):
    from concourse import bass_utils

    nc = build_nc_bypass()
    in_maps = make_in_maps(detect_labels, fact_table, conv_w, conv_b)
    res = bass_utils.run_bass_kernel_spmd(nc, in_maps, list(range(NCORES)))
    return assemble_output(res.results)

